# revision 1
# baseline (speedup 1.0000x reference)
"""Trainium2 Bass kernel for nn_ContrastiveEmbeddingLoss (N=8192, D=128).

Scheme ("column-attributed symmetric sums", v2):

Rows (anchors) are sharded 1024/core.  Labels are sorted on host so the
three classes {-1, 0, +1} occupy contiguous column ranges.  Key algebraic
facts exploited:

1. label-0 anchors have an empty negative set, so their loss is exactly 0.
   Their per-row sums are never needed => the entire class-0 COLUMN stripe
   is skipped (columns only; class-0 rows still feed other anchors' sums).

2. With the stabilizer bias b_i = -(o_i + O)/2 (o_i = 2||e_i||^2 >= row max
   by Cauchy-Schwarz, O = max_i o_i) and mask weights u_i = exp((o_i-O)/2):

       sum_i u_i * exp(sim_ij + b_i)  =  sum_i exp(sim_ij - O)

   which by symmetry of sim is the CLASS-RESOLVED ROW SUM of anchor j
   scaled by exp(o_j - O).  So per-class column sums computed by tiny
   [128x4] mask-matmuls on the TensorE (contraction over the partition
   axis) replace all row-direction accumulation: no reduce_max, no
   activation accumulators, no per-chunk rescaling.

Per core, per 128-row block: sim matmuls (lhsT = 2*bf16(E) block) fill
rotating [128,1024] PSUM slots; one wide ScalarE exp per slot (bf16 out);
one mask-matmul per 512-column window accumulates class sums into a
persistent PSUM region (pre-zeroed by zero-weight matmuls so start=False
accumulation is well-defined).  A separate diagonal mini-matmul recomputes
sim_rr bitwise-identically; its exp'd bf16 value is extracted with an
identity mask on the VectorE so the host can subtract the self term with
exact cancellation (the value subtracted is bit-identical to the addend
inside the PE accumulation, and f32 sums of non-negatives are monotone,
so P >= 0 always).

Device outputs per core: colp [16, COLW] f32 (class-partial column sums,
packed 4 partition-groups x 4-class rows) and dvecy [128, 8] f32 (exp'd
diagonal).  Host (f64): all-core reduce of colp (~100 KB), per-anchor
  S_c[j] = e^{O-o_j} * colp[c, j],   P = S_{c_j} + S_0 - self,
  G = S_{-c_j},  loss = log(P+G+eps) - log(P+eps),  mean over N.
"""

import numpy as np

N, D = 8192, 128
NCORES = 8
RPC = N // NCORES        # anchor rows per core
BPB = 128                # rows per block (= partition dim)
NBLK = RPC // BPB        # blocks per core
TEMPERATURE = 0.5
EPS = 1e-08
WIN = 512                # column window (= one PSUM bank of f32)
FILLW = 1024             # PSUM fill slot width (2 windows)

LAST_RESULT = None       # BassKernelResults of the most recent run (for test.py)


def _split_drain_tile_context(tile_mod, mybir, ScopedClock):
    """TileContext subclass that never emits more than one sync wait per
    instruction -- the walrus build here rejects any instruction carrying
    more than one ("Too many sync wait commands").  Excess waits are hoisted
    onto same-engine NoOp instructions inserted immediately before (engine
    program order makes sequential single waits equivalent to one multi-wait:
    a logical AND), and the tail drain is split into sequential drains."""

    class SplitWaitTileContext(tile_mod.TileContext):
        def _lower_ordered_insts(self, ordered):
            unassigned = mybir.EngineType.Unassigned
            for insts in ordered.values():
                new_list = []
                changed = False
                for inst in insts:
                    si = inst.sync_info
                    waits = list(si.on_wait) if si is not None and si.on_wait else []
                    eng = getattr(inst, "engine", None)
                    if len(waits) > 1 and eng is not None and eng != unassigned:
                        keep = [w for w in waits if w.sync_type != "semaphore"]
                        move = [w for w in waits if w.sync_type == "semaphore"]
                        if not keep and move:
                            keep = [move.pop()]
                        for w in move:
                            nop = mybir.InstNoOp(
                                name=f"I-{self.nc.next_id()}", ins=[], outs=[]
                            )
                            nop.engine = eng
                            nop.sync_info = mybir.SyncInfo(
                                on_wait=[w], on_update=[]
                            )
                            new_list.append(nop)
                        inst.sync_info = mybir.SyncInfo(
                            on_wait=keep,
                            on_update=list(si.on_update) if si.on_update else [],
                        )
                        changed = True
                    new_list.append(inst)
                if changed:
                    insts[:] = new_list
            return super()._lower_ordered_insts(ordered)

        def _drain_and_barrier(self, tick_clock, wait_clock):
            nc = self.nc
            drain_inst = nc.sync.drain()
            wait_clock.add_sem_waits(
                drain_inst.ins, ScopedClock({None: tick_clock.global_clock})
            )
            si = drain_inst.ins.sync_info
            waits = list(si.on_wait) if si is not None and si.on_wait else []
            if len(waits) > 1:
                drain_inst.ins.sync_info = mybir.SyncInfo(
                    on_wait=waits[:1],
                    on_update=list(si.on_update) if si.on_update else [],
                )
                for i in range(1, len(waits)):
                    extra = nc.sync.drain()
                    extra.ins.sync_info = mybir.SyncInfo(
                        on_wait=waits[i : i + 1], on_update=[]
                    )
            # Single-shot NEFF: skip the semaphore-clearing pass + second
            # barrier (cleanup for NEFF re-execution, which never happens
            # here -- each kernel() call compiles and runs a fresh NEFF).
            nc.all_engine_barrier()
            assert self.sems is not None
            popped = nc._tile_sem_poison_stack.pop()
            assert popped is self._sem_poison
            # Sems are intentionally NOT cleared or returned to the pool:
            # this is the outermost (only) TileContext of a one-shot program,
            # so nothing after it allocates semaphores.

    return SplitWaitTileContext


class _Sched:
    """Global (core-independent) column schedule.

    kept:  list of windows (gw, a, w, off, g, f):
      gw = global 512-window index, [a, a+w) = the kept (non-class-0)
      columns inside it, off = start of this window's columns in the packed
      etnz tensor, (g, f) = colp partition-group and free offset.
    fills: list of lists of kept-indices (<= 2 per fill, slot offsets
      512*pos within the fill).
    """

    def __init__(self, b1, b2):
        self.b1, self.b2 = b1, b2
        keep_ranges = [(0, b1), (b2, N)]
        kept = []
        off = 0
        for gw in range(N // WIN):
            lo, hi = gw * WIN, (gw + 1) * WIN
            for (ra, rb) in keep_ranges:
                a, b = max(lo, ra), min(hi, rb)
                if b > a:
                    kept.append([gw, a, b - a, off])
                    off += b - a
        # a 512-window can intersect both keep ranges only if the class-0
        # stripe is narrower than 512 columns; assert it can't happen
        gws = [k[0] for k in kept]
        assert len(set(gws)) == len(gws), "window split by narrow class-0"
        self.KC = off
        for i, k in enumerate(kept):
            k.append(i % 4)            # g: partition group
            k.append((i // 4) * WIN)   # f: colp free offset
        self.kept = kept
        self.KW = len(kept)
        self.COLW = WIN * ((self.KW + 3) // 4)
        self.fills = [list(range(i, min(i + 2, self.KW)))
                      for i in range(0, self.KW, 2)]


def _build_program(sched):
    from contextlib import ExitStack

    import concourse.bass as bass
    import concourse.mybir as mybir
    import concourse.tile as tile

    try:
        from bass_rust import ScopedClock
    except ImportError:
        from concourse.vector_clock import ScopedClock

    f32 = mybir.dt.float32
    bf16 = mybir.dt.bfloat16
    AF = mybir.ActivationFunctionType
    ALU = mybir.AluOpType
    X = mybir.AxisListType.X
    TC = _split_drain_tile_context(tile, mybir, ScopedClock)

    KC, KW, COLW = sched.KC, sched.KW, sched.COLW

    nc = bass.Bass("TRN2", target_bir_lowering=False, debug=False,
                   num_devices=NCORES)
    etnz_d = nc.dram_tensor("etnz", [D, KC], bf16, kind="ExternalInput").ap()
    e2o_d = nc.dram_tensor("et2own", [D, RPC], bf16, kind="ExternalInput").ap()
    edg_d = nc.dram_tensor("etdiag", [D, RPC], bf16, kind="ExternalInput").ap()
    wm_d = nc.dram_tensor("wmask", [BPB, NBLK * 4], bf16, kind="ExternalInput").ap()
    bs_d = nc.dram_tensor("biasb", [BPB, NBLK], f32, kind="ExternalInput").ap()
    id_d = nc.dram_tensor("ident", [BPB, BPB], bf16, kind="ExternalInput").ap()
    zr_d = nc.dram_tensor("zeros", [D, WIN], bf16, kind="ExternalInput").ap()
    colp_d = nc.dram_tensor("colp", [16, COLW], f32, kind="ExternalOutput").ap()
    dvy_d = nc.dram_tensor("dvecy", [BPB, NBLK], f32, kind="ExternalOutput").ap()

    with TC(nc) as tc, ExitStack() as ctx:
        singles = ctx.enter_context(tc.tile_pool(name="singles", bufs=1))
        ps = ctx.enter_context(tc.tile_pool(name="ps", bufs=1, space="PSUM"))
        scr = ctx.enter_context(tc.tile_pool(name="scr", bufs=1))

        # small tensors first (cheap, unblock early compute), etnz behind
        sb_bs = singles.tile([BPB, NBLK], f32)
        nc.sync.dma_start(out=sb_bs, in_=bs_d)
        sb_e2o = singles.tile([D, RPC], bf16)
        nc.sync.dma_start(out=sb_e2o, in_=e2o_d)
        sb_edg = singles.tile([D, RPC], bf16)
        nc.sync.dma_start(out=sb_edg, in_=edg_d)
        sb_wm = singles.tile([BPB, NBLK * 4], bf16)
        nc.sync.dma_start(out=sb_wm, in_=wm_d)
        sb_id = singles.tile([BPB, BPB], bf16)
        nc.sync.dma_start(out=sb_id, in_=id_d)
        sb_zr = singles.tile([D, WIN], bf16)
        nc.sync.dma_start(out=sb_zr, in_=zr_d)
        sb_et = singles.tile([D, KC], bf16)
        for a in range(0, KC, 1024):
            w = min(1024, KC - a)
            nc.sync.dma_start(out=sb_et[:, a:a + w], in_=etnz_d[:, a:a + w])

        dvy_sb = singles.tile([BPB, NBLK], f32)

        # persistent per-class column-sum accumulator, pre-zeroed via
        # zero-weight matmuls (start=True clears has_written; value 0)
        colpart = ps.tile([BPB, COLW], f32, tag="colpart")
        for z in range(COLW // WIN):
            nc.tensor.matmul(colpart[:, z * WIN:(z + 1) * WIN],
                             sb_zr[:, :BPB], sb_zr,
                             start=True, stop=False, skip_group_check=True)

        for b in range(NBLK):
            lhs = sb_e2o[:, b * BPB:(b + 1) * BPB]
            wmb = sb_wm[:, 4 * b:4 * b + 4]
            bias = sb_bs[:, b:b + 1]

            for fi, fill in enumerate(sched.fills):
                pf = ps.tile([BPB, FILLW], f32, tag="fill", bufs=2)
                yf = scr.tile([BPB, FILLW], bf16, tag="yf", bufs=3)
                # sim matmuls: one per kept window in this fill
                for pos, ki in enumerate(fill):
                    gw, a, w, off, g, f = sched.kept[ki]
                    nc.tensor.matmul(pf[:, pos * WIN:pos * WIN + w],
                                     lhs, sb_et[:, off:off + w],
                                     start=True, stop=True)
                # exp over the valid runs of this fill (merge when the
                # first window is full so its data abuts the second slot)
                runs = []
                for pos, ki in enumerate(fill):
                    w = sched.kept[ki][2]
                    if runs and runs[-1][0] + runs[-1][1] == pos * WIN:
                        runs[-1][1] += w
                    else:
                        runs.append([pos * WIN, w])
                for (ra, rw) in runs:
                    nc.scalar.activation(out=yf[:, ra:ra + rw],
                                         in_=pf[:, ra:ra + rw],
                                         func=AF.Exp, bias=bias, scale=1.0)
                # per-class column sums (contract over the 128 rows)
                for pos, ki in enumerate(fill):
                    gw, a, w, off, g, f = sched.kept[ki]
                    nc.tensor.matmul(
                        colpart[32 * g:32 * g + 4, f:f + w],
                        wmb, yf[:, pos * WIN:pos * WIN + w],
                        start=False, stop=(b == NBLK - 1 and ki == KW - 1),
                        skip_group_check=True, tile_position=(0, 32 * g))

            # diagonal: recompute sim_rr bitwise-identically, exp to bf16
            # (same rounding as the in-sum Y), extract with identity mask
            pd = ps.tile([BPB, BPB], f32, tag="dfill", bufs=1)
            nc.tensor.matmul(pd, lhs, sb_edg[:, b * BPB:(b + 1) * BPB],
                             start=True, stop=True)
            dy = scr.tile([BPB, BPB], bf16, tag="dy", bufs=2)
            nc.scalar.activation(out=dy, in_=pd, func=AF.Exp,
                                 bias=bias, scale=1.0)
            md = scr.tile([BPB, BPB], f32, tag="md", bufs=2)
            nc.vector.tensor_tensor(md, dy, sb_id, op=ALU.mult)
            nc.vector.reduce_sum(dvy_sb[:, b:b + 1], md, axis=X)

        # evacuate colpart and ship results
        colsb = singles.tile([BPB, COLW], f32)
        nc.vector.tensor_copy(colsb, colpart)
        for g in range(4):
            nc.sync.dma_start(out=colp_d[4 * g:4 * g + 4, :],
                              in_=colsb[32 * g:32 * g + 4, :])
        nc.sync.dma_start(out=dvy_d, in_=dvy_sb)

    return nc


def _host_prepare(labels, embeddings):
    """Sort by label, build the global schedule + per-core input maps."""
    import ml_dtypes

    labels = np.asarray(labels).astype(np.int64)
    emb = np.asarray(embeddings, dtype=np.float32)
    assert labels.shape == (N,) and emb.shape == (N, D)

    order = np.argsort(labels, kind="stable")
    lab_s = labels[order]
    b1 = int(np.searchsorted(lab_s, 0, side="left"))
    b2 = int(np.searchsorted(lab_s, 1, side="left"))
    assert 0 < b1 < b2 < N, "kernel assumes all three classes nonempty"
    assert b2 - b1 >= WIN, "kernel assumes class-0 stripe >= one window"

    sched = _Sched(b1, b2)

    eb16 = emb[order].astype(ml_dtypes.bfloat16)          # [N, D] bf16
    ebf = eb16.astype(np.float32)
    et = np.ascontiguousarray(ebf.T).astype(ml_dtypes.bfloat16)  # [D, N]
    et2 = (et.astype(np.float32) * 2.0).astype(ml_dtypes.bfloat16)  # exact

    o = 2.0 * (ebf.astype(np.float64) ** 2).sum(axis=1)   # [N] f64
    O = float(o.max())
    bias = (-(o + O) / 2.0).astype(np.float32)
    u16 = np.exp((o - O) / 2.0).astype(np.float32).astype(ml_dtypes.bfloat16)
    cls = (lab_s + 1).astype(np.int64)                    # 0,1,2

    etnz = np.empty((D, sched.KC), dtype=ml_dtypes.bfloat16)
    for (gw, a, w, off, g, f) in sched.kept:
        etnz[:, off:off + w] = et[:, a:a + w]
    etnz = np.ascontiguousarray(etnz)

    in_maps = []
    for c in range(NCORES):
        rows = slice(c * RPC, (c + 1) * RPC)
        wm = np.zeros((BPB, NBLK * 4), np.float32)
        for b in range(NBLK):
            rr = np.arange(c * RPC + b * BPB, c * RPC + (b + 1) * BPB)
            wm[np.arange(BPB), 4 * b + cls[rr]] = u16[rr].astype(np.float32)
        in_maps.append({
            "etnz": etnz,
            "et2own": np.ascontiguousarray(et2[:, rows]),
            "etdiag": np.ascontiguousarray(et[:, rows]),
            "wmask": wm.astype(ml_dtypes.bfloat16),
            "biasb": np.ascontiguousarray(
                bias[rows].reshape(NBLK, BPB).T),
            "ident": np.eye(BPB, dtype=np.float32).astype(ml_dtypes.bfloat16),
            "zeros": np.zeros((D, WIN), np.float32).astype(ml_dtypes.bfloat16),
        })

    host = {
        "order": order, "lab_s": lab_s, "cls": cls, "b1": b1, "b2": b2,
        "o": o, "O": O, "u32": u16.astype(np.float32), "sched": sched,
    }
    return sched, in_maps, host


def _host_epilogue(host, colps, dvecys):
    """Combine per-core partials into the scalar mean loss (f64)."""
    sched = host["sched"]
    cls, o, O, u32 = host["cls"], host["o"], host["O"], host["u32"]

    # f32 monotone reduction across cores preserves sum >= self-term
    colp = np.zeros_like(colps[0], dtype=np.float32)
    for cp in colps:
        colp = colp + cp.astype(np.float32)

    # per-class sums S3[c, j] (still scaled by exp(-O)), j in sorted space
    S3 = np.zeros((3, N), np.float32)
    valid = np.zeros(N, bool)
    for (gw, a, w, off, g, f) in sched.kept:
        for c in range(3):
            S3[c, a:a + w] = colp[4 * g + c, f:f + w]
        valid[a:a + w] = True

    # exp'd diagonal per sorted anchor (bf16 value as f32)
    dvy = np.concatenate(
        [np.asarray(d, np.float32).T.reshape(-1) for d in dvecys])  # [N]

    j = np.nonzero(valid)[0]                      # all +-1 anchors
    cj = cls[j]
    selfp = (u32[j] * dvy[j]).astype(np.float32)  # exact f32 product
    own = S3[cj, j]                               # includes the self term
    ppre = np.maximum(own.astype(np.float32) - selfp, np.float32(0.0))
    pos = ppre.astype(np.float64) + S3[1, j].astype(np.float64)
    neg = S3[2 - cj, j].astype(np.float64)
    scale = np.exp(O - o[j])                      # f64, may be huge
    P = scale * pos
    G = scale * neg
    loss = np.log(P + G + EPS) - np.log(P + EPS)
    return np.float32(loss.sum() / N)


def _ensure_ntff_hook():
    """Register a stand-in ``antenv.axon_hooks`` if the image lacks it.

    ``run_bass_kernel_spmd(trace=True)`` under axon imports
    ``antenv.axon_hooks.get_axon_ntff_profile_hook`` unguarded; this image's
    ``antenv`` has no ``axon_hooks`` submodule, so tracing would crash.
    Provide the hook via direct ctypes calls into libaxon_pjrt.so (same C ABI
    the boot shim uses); if the .so or symbols are missing the getter returns
    None and concourse degrades to running without a trace."""
    import contextlib
    import ctypes
    import sys
    import types

    try:
        import antenv.axon_hooks  # noqa: F401
        return
    except ImportError:
        pass

    mod = types.ModuleType("antenv.axon_hooks")
    holder = [None]
    mod.set_axon_ntff_profile_hook = lambda h: holder.__setitem__(0, h)
    mod.get_axon_ntff_profile_hook = lambda: holder[0]

    try:
        lib = ctypes.CDLL("/opt/axon/libaxon_pjrt.so")
        if hasattr(lib, "axon_start_nrt_profile"):
            lib.axon_start_nrt_profile.argtypes = [
                ctypes.POINTER(ctypes.c_int64), ctypes.c_size_t]
            lib.axon_start_nrt_profile.restype = ctypes.c_int64
            lib.axon_stop_nrt_profile.argtypes = [ctypes.c_char_p]
            lib.axon_stop_nrt_profile.restype = ctypes.c_int64

            @contextlib.contextmanager
            def _hook(output_dir, device_ids):
                import jax
                jax.devices()
                if device_ids:
                    ids = (ctypes.c_int64 * len(device_ids))(*device_ids)
                    rc = lib.axon_start_nrt_profile(ids, len(device_ids))
                else:
                    rc = lib.axon_start_nrt_profile(None, 0)
                if rc != 0:
                    raise RuntimeError(f"axon_start_nrt_profile rc={rc}")
                try:
                    yield
                finally:
                    n = lib.axon_stop_nrt_profile(str(output_dir).encode())
                    if n < 0:
                        raise RuntimeError(f"axon_stop_nrt_profile rc={n}")

            holder[0] = _hook
    except OSError:
        pass

    sys.modules["antenv.axon_hooks"] = mod
    try:
        import antenv
        antenv.axon_hooks = mod
    except ImportError:
        pass


def kernel(labels, embeddings, **_unused):
    global LAST_RESULT
    _ensure_ntff_hook()
    from concourse.bass_utils import run_bass_kernel_spmd

    sched, in_maps, host = _host_prepare(labels, embeddings)
    nc = _build_program(sched)
    res = run_bass_kernel_spmd(nc, in_maps, core_ids=list(range(NCORES)))
    LAST_RESULT = res

    colps = [res.results[i]["colp"] for i in range(NCORES)]
    dvecys = [res.results[i]["dvecy"] for i in range(NCORES)]
    return np.array(_host_epilogue(host, colps, dvecys), dtype=np.float32)



# revision 16
# speedup vs baseline: 1.0552x; 1.0552x over previous
"""Trainium2 Bass kernel for nn_ContrastiveEmbeddingLoss (N=8192, D=128).

Scheme ("per-class block triangle", v3):

Labels are sorted on host; classes {-1, 0, +1}.  Only +-1 anchors have
nonzero loss; label-0 anchors contribute exactly 0 (their negative set is
empty).  With the global stabilizer O = max_i o_i (o_i = 2||e_i||^2 >=
any sim row max by Cauchy-Schwarz), every needed quantity is a sum of
    Y_ij = exp(sim_ij - O)
over column groups, so each unordered pair {i,j} needs to be exp'd ONCE
and can be attributed to both sides (the matrix is symmetric).

Pair coverage (each +-1 class split into KB=24 row blocks of height
h<=128; dead lanes are free because engine cost depends on the free
dim only):

  * within-class pairs: wrapped block-diagonal cover; row block r
    processes column blocks (r+p) mod 24, p=0..12.  p=0 is the self
    block (diagonal killed in PSUM by a -BIG*I accumulate matmul);
    p=1..11 tiles are mirrored to the partner side by accumulating Y
    into a column accumulator Z (DVE); p=12 tiles are processed by BOTH
    end blocks (row-attributed twice), which keeps every slot at exactly
    13 positions -> identical program on all 8 cores.
  * (-1,+1) pairs: full rectangle on the -1 rows ("opp" section);
    mirrored to the +1 side via a Z accumulator (GpSimd).
  * (+-1, 0) pairs: full rectangle on the +-1 rows ("zero" section);
    the 0-side needs nothing.

Per (slot, chunk<=1024 cols): PE sim matmul (lhsT = 2*bf16(E) rows of
the block) into rotating PSUM; one ScalarE exp -> Y bf16 (uniform bias
-O for real lanes, -1e30 for dead lanes); DVE reduce_sum -> one f32
strip entry (per-anchor partial row sums, class-pure by construction);
DVE/GpSimd Y-accumulate into Z (bf16).  Z is partition-reduced at the
end by ones-vector matmuls into PSUM and DMA'd out; strips are DMA'd
raw ([128, ~45] f32) and combined on host in f64.

Dead (padding) columns inside class sections all carry e=0, so their
Y value is the single number v = table_exp(-O); a dedicated 16-wide
all-dead run at the end of the zero section measures 16*v exactly, and
the host subtracts the known dead-column counts * v from affected strip
entries.  The diagonal is excluded on device, so every per-anchor sum
is a sum of non-negatives: no catastrophic cancellation anywhere.

Host (f64): P = T_same + T_zero + S_same, G = T_opp + S_opp,
loss = logaddexp(logaddexp(lP, lG), leps) - logaddexp(lP, leps) with
lX = ln(X) + O - o_a, leps = ln(1e-8); mean over all N anchors.
"""

import numpy as np

N, D = 8192, 128
NCORES = 8
KB = 24                 # row blocks per +-1 class (3 per core)
TEMPERATURE = 0.5
EPS = 1e-08
CHUNK = 1024            # psum fill width (2 banks f32)
MMW = 512               # max matmul piece width
DEADRUN = 16            # all-dead measuring run at the end of zero sec
BIG = 1e30

LAST_RESULT = None      # BassKernelResults of the most recent run


# ---------------------------------------------------------------------------
# schedule (shared by device builder, emulator and host epilogue)

def _split(lo, hi, step):
    return [(a, min(a + step, hi)) for a in range(lo, hi, step)]


class Sched:
    """All program structure derived from (n1, n0, n2).

    Local Et layout: [A-sec 15*h1 | B-sec 15*h2 | opp 24*h2 | zero n0+16].
    A-sec of core c holds class -1 blocks (3c+j) mod 24, j=0..14 (each
    h1 wide, zero-filled past the block's real rows); same for B-sec with
    class +1.  opp = full class +1 in plain block order; zero = class 0
    rows followed by DEADRUN zero columns.

    Slots (uniform on every core): k=0..2 -> A row block 3c+k,
    k=3..5 -> B row block 3c+(k-3).  Slot sections:
      A slot: span [k*h1, (k+13)*h1) in A-sec, opp, zero
      B slot: span [j*h2, (j+13)*h2) in B-sec, zero
    """

    def __init__(self, n1, n0, n2):
        assert n1 >= 1 and n0 >= 1 and n2 >= 1
        self.n1, self.n0, self.n2 = n1, n0, n2
        self.h1 = -(-n1 // KB)
        self.h2 = -(-n2 // KB)
        assert self.h1 <= 128 and self.h2 <= 128
        h1, h2 = self.h1, self.h2
        self.offA = 0
        self.offB = 15 * h1
        self.offO = self.offB + 15 * h2
        self.offZ = self.offO + KB * h2
        self.WZ = n0 + DEADRUN
        self.LW = self.offZ + self.WZ

        # chunks: list of dicts. phase 0 = spans, 1 = opps, 2 = zeros
        # (so Z finishes early and its reduce overlaps the zero phase).
        self.chunks = []
        for k in range(6):
            isA = k < 3
            j, h, off = (k, h1, self.offA) if isA else (k - 3, h2, self.offB)
            for (lo, hi) in _split(j * h, (j + 13) * h, CHUNK):
                self.chunks.append(dict(
                    slot=k, sec="span", phase=0, lo=off + lo, hi=off + hi,
                    slo=lo - j * h, kill=(lo == j * h)))
        for k in range(3):
            for (lo, hi) in _split(0, KB * h2, CHUNK):
                self.chunks.append(dict(
                    slot=k, sec="opp", phase=1, lo=self.offO + lo,
                    hi=self.offO + hi, slo=lo, kill=False))
        for k in range(6):
            for (lo, hi) in _split(0, n0, CHUNK) + [(n0, n0 + DEADRUN)]:
                self.chunks.append(dict(
                    slot=k, sec="zero", phase=2, lo=self.offZ + lo,
                    hi=self.offZ + hi, slo=lo, kill=False))
        self.chunks.sort(key=lambda ch: (ch["phase"], ch["slot"], ch["lo"]))
        for i, ch in enumerate(self.chunks):
            ch["entry"] = i
        self.nstrip = len(self.chunks)

        # Z accumulate ops per chunk: (ztgt, z0, y0, y1)
        # A/B spans: positions p=1..11 -> Z col (k+p-1)*h + off  [DVE]
        # opp: all cols -> ZO [GpSimd]
        self.WZA = -(-13 * h1 // MMW) * MMW
        self.WZB = -(-13 * h2 // MMW) * MMW
        self.WZO = -(-KB * h2 // MMW) * MMW
        for ch in self.chunks:
            k, ch_w = ch["slot"], ch["hi"] - ch["lo"]
            ops = []
            if ch["sec"] == "span":
                isA = k < 3
                j, h = (k, h1) if isA else (k - 3, h2)
                s0, s1 = ch["slo"], ch["slo"] + ch_w     # span-local range
                a, b = max(s0, h), min(s1, 12 * h)       # positions 1..11
                if b > a:
                    ops.append(("ZA" if isA else "ZB",
                                j * h + (a - h), a - s0, b - s0))
            elif ch["sec"] == "opp":
                ops.append(("ZO", ch["slo"], 0, ch_w))
            ch["zops"] = ops

        # Z accumulators are DMA'd out raw (bf16) as soon as their last
        # add lands; the 128-partition sum happens on host.
        self.WZ_ALL = self.WZA + self.WZB + self.WZO


# ---------------------------------------------------------------------------
# device program

def _split_drain_tile_context(tile_mod, mybir, ScopedClock):
    """TileContext subclass that never emits more than one sync wait per
    instruction -- this walrus build rejects any instruction carrying
    more than one ("Too many sync wait commands").  Excess waits are hoisted
    onto same-engine NoOp instructions inserted immediately before, and the
    tail drain is split into sequential drains."""

    class SplitWaitTileContext(tile_mod.TileContext):
        def _lower_ordered_insts(self, ordered):
            unassigned = mybir.EngineType.Unassigned
            for insts in ordered.values():
                new_list = []
                changed = False
                for inst in insts:
                    si = inst.sync_info
                    waits = list(si.on_wait) if si is not None and si.on_wait else []
                    eng = getattr(inst, "engine", None)
                    if len(waits) > 1 and eng is not None and eng != unassigned:
                        keep = [w for w in waits if w.sync_type != "semaphore"]
                        move = [w for w in waits if w.sync_type == "semaphore"]
                        if not keep and move:
                            keep = [move.pop()]
                        for w in move:
                            nop = mybir.InstNoOp(
                                name=f"I-{self.nc.next_id()}", ins=[], outs=[]
                            )
                            nop.engine = eng
                            nop.sync_info = mybir.SyncInfo(
                                on_wait=[w], on_update=[]
                            )
                            new_list.append(nop)
                        inst.sync_info = mybir.SyncInfo(
                            on_wait=keep,
                            on_update=list(si.on_update) if si.on_update else [],
                        )
                        changed = True
                    new_list.append(inst)
                if changed:
                    insts[:] = new_list
            return super()._lower_ordered_insts(ordered)

        def _drain_and_barrier(self, tick_clock, wait_clock):
            nc = self.nc
            drain_inst = nc.sync.drain()
            wait_clock.add_sem_waits(
                drain_inst.ins, ScopedClock({None: tick_clock.global_clock})
            )
            si = drain_inst.ins.sync_info
            waits = list(si.on_wait) if si is not None and si.on_wait else []
            if len(waits) > 1:
                drain_inst.ins.sync_info = mybir.SyncInfo(
                    on_wait=waits[:1],
                    on_update=list(si.on_update) if si.on_update else [],
                )
                for i in range(1, len(waits)):
                    extra = nc.sync.drain()
                    extra.ins.sync_info = mybir.SyncInfo(
                        on_wait=waits[i : i + 1], on_update=[]
                    )
            # Single-shot NEFF: skip the semaphore-clearing pass + second
            # barrier (cleanup for NEFF re-execution, which never happens
            # here).
            nc.all_engine_barrier()
            assert self.sems is not None
            popped = nc._tile_sem_poison_stack.pop()
            assert popped is self._sem_poison
            # Sems intentionally not cleared/returned: outermost (only)
            # TileContext of a one-shot program.

    return SplitWaitTileContext


def _build_program(s: Sched):
    from contextlib import ExitStack

    import concourse.bass as bass
    import concourse.mybir as mybir
    import concourse.tile as tile

    try:
        from bass_rust import ScopedClock
    except ImportError:
        from concourse.vector_clock import ScopedClock

    f32 = mybir.dt.float32
    bf16 = mybir.dt.bfloat16
    AF = mybir.ActivationFunctionType
    ALU = mybir.AluOpType
    X = mybir.AxisListType.X
    TC = _split_drain_tile_context(tile, mybir, ScopedClock)

    nc = bass.Bass("TRN2", target_bir_lowering=False, debug=False,
                   num_devices=NCORES)
    etl_d = nc.dram_tensor("etl", [D, s.LW], bf16, kind="ExternalInput").ap()
    lhs_d = nc.dram_tensor("lhs", [D, 6 * 128], bf16, kind="ExternalInput").ap()
    bias_d = nc.dram_tensor("bias", [128, 6], f32, kind="ExternalInput").ap()
    i128_d = nc.dram_tensor("i128", [128, 128], bf16, kind="ExternalInput").ap()
    k128_d = nc.dram_tensor("k128", [128, 128], bf16, kind="ExternalInput").ap()
    strips_d = nc.dram_tensor("strips", [128, s.nstrip], f32,
                              kind="ExternalOutput").ap()
    zraw_d = nc.dram_tensor("zraw", [128, s.WZ_ALL], bf16,
                            kind="ExternalOutput").ap()

    with TC(nc) as tc, ExitStack() as ctx:
        singles = ctx.enter_context(tc.tile_pool(name="singles", bufs=1))
        ps = ctx.enter_context(tc.tile_pool(name="ps", bufs=1, space="PSUM"))
        scr = ctx.enter_context(tc.tile_pool(name="scr", bufs=1))

        # small inputs first so compute can start as soon as possible
        sb_lhs = singles.tile([D, 6 * 128], bf16)
        nc.sync.dma_start(out=sb_lhs, in_=lhs_d)
        sb_bias = singles.tile([128, 6], f32)
        nc.sync.dma_start(out=sb_bias, in_=bias_d)
        sb_i = singles.tile([128, 128], bf16)
        nc.sync.dma_start(out=sb_i, in_=i128_d)
        sb_k = singles.tile([128, 128], bf16)
        nc.sync.dma_start(out=sb_k, in_=k128_d)
        sb_et = singles.tile([D, s.LW], bf16)
        for a in range(0, s.LW, 2048):
            w = min(2048, s.LW - a)
            nc.sync.dma_start(out=sb_et[:, a:a + w], in_=etl_d[:, a:a + w])

        strips = singles.tile([128, s.nstrip], f32)
        ZA = singles.tile([128, s.WZA], bf16)
        ZB = singles.tile([128, s.WZB], bf16)
        ZO = singles.tile([128, s.WZO], bf16)
        zmap = {"ZA": ZA, "ZB": ZB, "ZO": ZO}
        nc.gpsimd.memset(ZA, 0.0)
        nc.gpsimd.memset(ZB, 0.0)
        nc.gpsimd.memset(ZO, 0.0)

        prev_phase = 0
        for ch in s.chunks:
            if ch["phase"] == 1 and prev_phase == 0:
                # spans done: ZA/ZB final -> ship them (overlaps phases 1-2)
                nc.sync.dma_start(out=zraw_d[:, 0:s.WZA], in_=ZA)
                nc.sync.dma_start(out=zraw_d[:, s.WZA:s.WZA + s.WZB], in_=ZB)
            if ch["phase"] == 2 and prev_phase == 1:
                nc.sync.dma_start(out=zraw_d[:, s.WZA + s.WZB:], in_=ZO)
            prev_phase = ch["phase"]
            k, w = ch["slot"], ch["hi"] - ch["lo"]
            lhs = sb_lhs[:, 128 * k:128 * (k + 1)]
            h = s.h1 if k < 3 else s.h2
            pf = ps.tile([128, CHUNK], f32, tag="fill", bufs=4)
            pieces = _split(0, w, MMW)
            for (a, b) in pieces:
                last = (b == w) and not ch["kill"]
                nc.tensor.matmul(pf[:, a:b], lhs,
                                 sb_et[:, ch["lo"] + a:ch["lo"] + b],
                                 start=True, stop=last,
                                 skip_group_check=True)
            if ch["kill"]:
                # diagonal killer: psum[:, :h] += -BIG * I
                nc.tensor.matmul(pf[:, 0:h], sb_k, sb_i[:, 0:h],
                                 start=False, stop=True,
                                 skip_group_check=True)
            yf = scr.tile([128, CHUNK], bf16, tag="yf", bufs=3)
            nc.scalar.activation(out=yf[:, 0:w], in_=pf[:, 0:w],
                                 func=AF.Exp, bias=sb_bias[:, k:k + 1],
                                 scale=1.0)
            e = ch["entry"]
            nc.vector.reduce_sum(strips[:, e:e + 1], yf[:, 0:w], axis=X)
            for (zt, z0, y0, y1) in ch["zops"]:
                Z = zmap[zt]
                eng = nc.gpsimd if zt == "ZO" else nc.vector
                eng.tensor_tensor(Z[:, z0:z0 + (y1 - y0)],
                                  Z[:, z0:z0 + (y1 - y0)],
                                  yf[:, y0:y1], op=ALU.add)

        nc.sync.dma_start(out=strips_d, in_=strips)

    return nc


# ---------------------------------------------------------------------------
# host preparation

def _host_prepare(labels, embeddings):
    import ml_dtypes

    labels = np.asarray(labels).astype(np.int64)
    emb = np.asarray(embeddings, dtype=np.float32)
    assert labels.shape == (N,) and emb.shape == (N, D)

    order = np.argsort(labels, kind="stable")
    lab_s = labels[order]
    b1 = int(np.searchsorted(lab_s, 0, side="left"))
    b2 = int(np.searchsorted(lab_s, 1, side="left"))
    n1, n0, n2 = b1, b2 - b1, N - b2
    s = Sched(n1, n0, n2)

    eb16 = emb[order].astype(ml_dtypes.bfloat16)
    ebf = eb16.astype(np.float32)                    # sorted, bf16-rounded
    o = 2.0 * (ebf.astype(np.float64) ** 2).sum(axis=1)
    O = float(o.max())

    et = np.ascontiguousarray(ebf.T)                 # [D, N] f32 of bf16 vals
    rows1 = et[:, 0:b1]                              # class -1 columns
    rows0 = et[:, b1:b2]
    rows2 = et[:, b2:N]

    def blockpack(cls_cols, h, blks):
        """[D, len(blks)*h] with the given class blocks, zero-padded."""
        n = cls_cols.shape[1]
        out = np.zeros((D, len(blks) * h), np.float32)
        for i, b in enumerate(blks):
            a, e = b * h, min((b + 1) * h, n)
            if e > a:
                out[:, i * h:i * h + (e - a)] = cls_cols[:, a:e]
        return out

    h1, h2 = s.h1, s.h2
    in_maps = []
    for c in range(NCORES):
        ablks = [(3 * c + j) % KB for j in range(15)]
        etl = np.concatenate([
            blockpack(rows1, h1, ablks),
            blockpack(rows2, h2, ablks),
            blockpack(rows2, h2, list(range(KB))),
            np.pad(rows0, ((0, 0), (0, DEADRUN))),
        ], axis=1)
        assert etl.shape[1] == s.LW
        lhs = np.zeros((D, 6 * 128), np.float32)
        bias = np.full((128, 6), -BIG, np.float32)
        for k in range(6):
            isA = k < 3
            rows, h, nn = (rows1, h1, n1) if isA else (rows2, h2, n2)
            b = 3 * c + (k if isA else k - 3)
            a, e = b * h, min((b + 1) * h, nn)
            if e > a:
                lhs[:, 128 * k:128 * k + (e - a)] = 2.0 * rows[:, a:e]
                bias[0:e - a, k] = np.float32(-O)
        in_maps.append({
            "etl": etl.astype(ml_dtypes.bfloat16),
            "lhs": lhs.astype(ml_dtypes.bfloat16),
            "bias": bias,
            "i128": np.eye(128, dtype=np.float32).astype(ml_dtypes.bfloat16),
            "k128": (-BIG * np.eye(128, dtype=np.float32)).astype(
                ml_dtypes.bfloat16),
        })

    host = dict(order=order, lab_s=lab_s, n1=n1, n0=n0, n2=n2,
                o=o, O=O, s=s)
    return s, in_maps, host


# ---------------------------------------------------------------------------
# host epilogue

def _host_epilogue(host, strips_all, zred_all):
    s: Sched = host["s"]
    n1, n0, n2 = host["n1"], host["n0"], host["n2"]
    o, O = host["o"], host["O"]
    h1, h2 = s.h1, s.h2

    def realrows(isA, b):
        nn, h = (n1, h1) if isA else (n2, h2)
        return max(0, min((b + 1) * h, nn) - b * h)

    # per-anchor accumulators in class-local index space
    T_same = [np.zeros(n1), np.zeros(n2)]
    T_opp = [np.zeros(n1), np.zeros(n2)]
    T_zero = [np.zeros(n1), np.zeros(n2)]
    S_same = [np.zeros(n1), np.zeros(n2)]
    S_opp = np.zeros(n2)

    # the dead-column unit value v per core (from the all-dead run of
    # slot 0; lane 0 is always real since block 3c has >= 1 real row)
    deadrun_entry = {}
    for ch in s.chunks:
        if ch["sec"] == "zero" and ch["slo"] == n0:
            deadrun_entry[ch["slot"]] = ch["entry"]

    for c in range(NCORES):
        st = np.asarray(strips_all[c], np.float64)
        v = st[0, deadrun_entry[0]] / DEADRUN
        for ch in s.chunks:
            k, e = ch["slot"], ch["entry"]
            isA = k < 3
            b = 3 * c + (k if isA else k - 3)
            nr = realrows(isA, b)
            if nr == 0:
                continue
            vals = st[0:nr, e].copy()
            w = ch["hi"] - ch["lo"]
            cls_i = 0 if isA else 1
            if ch["sec"] == "span":
                h = h1 if isA else h2
                nn = n1 if isA else n2
                # dead columns: positions overlapping short blocks
                s0, s1 = ch["slo"], ch["slo"] + w
                ndead = 0
                for p in range(s0 // h, (s1 - 1) // h + 1):
                    pb = (b + p) % KB
                    pr = realrows(isA, pb)
                    # dead cols of position p: [p*h + pr, (p+1)*h)
                    a0, a1 = max(s0, p * h + pr), min(s1, (p + 1) * h)
                    ndead += max(0, a1 - a0)
                vals -= ndead * v
                T_same[cls_i][b * h:b * h + nr] += vals
            elif ch["sec"] == "opp":
                a0, a1 = max(ch["slo"], n2), min(ch["slo"] + w, KB * h2)
                vals -= max(0, a1 - a0) * v
                T_opp[cls_i][b * h1:b * h1 + nr] += vals
            else:  # zero
                if ch["slo"] >= n0:
                    continue  # dead run
                h = h1 if isA else h2
                T_zero[cls_i][b * h:b * h + nr] += vals

        zr = np.asarray(zred_all[c], np.float32).astype(np.float64)
        zsum = zr.sum(axis=0)                     # partition reduce (host)
        zsA = zsum[0:s.WZA]
        zsB = zsum[s.WZA:s.WZA + s.WZB]
        zsO = zsum[s.WZA + s.WZB:]
        for zs, isA in ((zsA, True), (zsB, False)):
            h, nn, cls_i = (h1, n1, 0) if isA else (h2, n2, 1)
            z = np.arange(13 * h)
            blk = (3 * c + z // h + 1) % KB
            off = z % h
            gi = blk * h + off
            rr = np.minimum((blk + 1) * h, nn) - blk * h
            m = (off < rr) & (gi < nn)
            np.add.at(S_same[cls_i], gi[m], zs[z[m]])
        S_opp += zsO[0:n2]

    leps = np.log(EPS)
    total = 0.0
    for cls_i, nn, base in ((0, n1, 0), (1, n2, n1 + n0)):
        P = np.maximum(T_same[cls_i] + T_zero[cls_i] + S_same[cls_i], 0.0)
        G = np.maximum(T_opp[cls_i] + (S_opp if cls_i == 1 else 0.0), 0.0)
        shift = O - o[base:base + nn]            # sorted-space o
        with np.errstate(divide="ignore"):
            lP = np.where(P > 0, np.log(np.maximum(P, 1e-300)), -np.inf) + shift
            lG = np.where(G > 0, np.log(np.maximum(G, 1e-300)), -np.inf) + shift
        loss = (np.logaddexp(np.logaddexp(lP, lG), leps)
                - np.logaddexp(lP, leps))
        total += loss.sum()
    return np.float32(total / N)


# ---------------------------------------------------------------------------
# numpy emulation of one core (for fast correctness checking)

def _emulate_core(s: Sched, im):
    import ml_dtypes

    etl = np.asarray(im["etl"], np.float32)
    lhs = np.asarray(im["lhs"], np.float32)
    bias = np.asarray(im["bias"], np.float32)

    strips = np.zeros((128, s.nstrip), np.float32)
    Z = {"ZA": np.zeros((128, s.WZA), ml_dtypes.bfloat16),
         "ZB": np.zeros((128, s.WZB), ml_dtypes.bfloat16),
         "ZO": np.zeros((128, s.WZO), ml_dtypes.bfloat16)}
    for ch in s.chunks:
        k, w = ch["slot"], ch["hi"] - ch["lo"]
        h = s.h1 if k < 3 else s.h2
        L = lhs[:, 128 * k:128 * (k + 1)]
        sim = (L.T @ etl[:, ch["lo"]:ch["hi"]]).astype(np.float32)
        if ch["kill"]:
            sim[:, 0:h] += -BIG * np.eye(128, dtype=np.float32)[:, 0:h]
        y = np.exp(sim + bias[:, k:k + 1]).astype(ml_dtypes.bfloat16)
        yf = y.astype(np.float32)
        strips[:, ch["entry"]] = yf.sum(axis=1, dtype=np.float32)
        for (zt, z0, y0, y1) in ch["zops"]:
            zv = Z[zt][:, z0:z0 + (y1 - y0)].astype(np.float32)
            Z[zt][:, z0:z0 + (y1 - y0)] = (zv + yf[:, y0:y1]).astype(
                ml_dtypes.bfloat16)
    zraw = np.concatenate([Z["ZA"], Z["ZB"], Z["ZO"]], axis=1)
    return strips, zraw


# ---------------------------------------------------------------------------
# axon NTFF hook shim (unchanged from v1)

def _ensure_ntff_hook():
    """Register a stand-in ``antenv.axon_hooks`` if the image lacks it."""
    import contextlib
    import ctypes
    import sys
    import types

    try:
        import antenv.axon_hooks  # noqa: F401
        return
    except ImportError:
        pass

    mod = types.ModuleType("antenv.axon_hooks")
    holder = [None]
    mod.set_axon_ntff_profile_hook = lambda h: holder.__setitem__(0, h)
    mod.get_axon_ntff_profile_hook = lambda: holder[0]

    try:
        lib = ctypes.CDLL("/opt/axon/libaxon_pjrt.so")
        if hasattr(lib, "axon_start_nrt_profile"):
            lib.axon_start_nrt_profile.argtypes = [
                ctypes.POINTER(ctypes.c_int64), ctypes.c_size_t]
            lib.axon_start_nrt_profile.restype = ctypes.c_int64
            lib.axon_stop_nrt_profile.argtypes = [ctypes.c_char_p]
            lib.axon_stop_nrt_profile.restype = ctypes.c_int64

            @contextlib.contextmanager
            def _hook(output_dir, device_ids):
                import jax
                jax.devices()
                if device_ids:
                    ids = (ctypes.c_int64 * len(device_ids))(*device_ids)
                    rc = lib.axon_start_nrt_profile(ids, len(device_ids))
                else:
                    rc = lib.axon_start_nrt_profile(None, 0)
                if rc != 0:
                    raise RuntimeError(f"axon_start_nrt_profile rc={rc}")
                try:
                    yield
                finally:
                    n = lib.axon_stop_nrt_profile(str(output_dir).encode())
                    if n < 0:
                        raise RuntimeError(f"axon_stop_nrt_profile rc={n}")

            holder[0] = _hook
    except OSError:
        pass

    sys.modules["antenv.axon_hooks"] = mod
    try:
        import antenv
        antenv.axon_hooks = mod
    except ImportError:
        pass


# ---------------------------------------------------------------------------

def kernel(labels, embeddings, **_unused):
    global LAST_RESULT
    _ensure_ntff_hook()
    from concourse.bass_utils import run_bass_kernel_spmd

    s, in_maps, host = _host_prepare(labels, embeddings)
    nc = _build_program(s)
    res = run_bass_kernel_spmd(nc, in_maps, core_ids=list(range(NCORES)))
    LAST_RESULT = res

    strips_all = [res.results[i]["strips"] for i in range(NCORES)]
    zred_all = [res.results[i]["zraw"] for i in range(NCORES)]
    return np.array(_host_epilogue(host, strips_all, zred_all),
                    dtype=np.float32)
